# revision 21
# baseline (speedup 1.0000x reference)
"""Trainium2 Bass kernel for nn_CPDFP_25701084299789 (pooling).

Reference math (B=64, C=256, H=W=32), per branch x in {x1, x2}:
    center  = x[:, :, 16, 16]                               (B, C)
    dot     = sum_c(x * center) / C                         (B, 1, H, W)
    attn    = sigmoid(conv_w @ concat([x, dot], ch))        (B, C, H, W)
    pool    = sum_hw(x * attn) / (sum_hw(attn) + 1e-8)      (B, C)
output = pool(x1) + pool(x2)   # the caLayer softmax is over a size-1
                               # axis == 1.0, so it contributes nothing.

Key algebraic simplification: the concat+1x1-conv is a rank-1 weight
update, Y_b = (W[:, :C] + w_last (outer) center_b / C) @ X_b, so no
extra matmuls for the `dot` channel are needed.

Sharding: data-parallel over batch across 8 cores; each core handles
8 batch items x 2 branches = 16 independent (256, 1024) items.
Params (conv_w-derived) replicated.
"""

import os
import threading
from contextlib import ExitStack

import numpy as np

import concourse.bacc as bacc
import concourse.mybir as mybir
import concourse.tile as tile
from concourse.bass_utils import run_bass_kernel_spmd

N_CORES = 8
B, C, HW = 64, 256, 1024          # batch, channels, H*W
B_LOC = B // N_CORES              # batch items per core
ITEMS = 2 * B_LOC                 # branch-items per core (x1 + x2)
CENTER = 16 * 32 + 16             # flat index of (h//2, w//2)
FP = mybir.dt.float32
FPR = mybir.dt.float32r           # same bits; PE runs 4x faster than fp32
BF = mybir.dt.bfloat16

# bf16 x-path: halves HBM traffic at the cost of ~bf16-level accuracy
# (rel err ~2e-3 vs f32r's ~1e-4) for only a ~10% modeled speedup (ACT/DVE
# see no bf16 gain on this op mix). Off by default.
USE_BF16 = os.environ.get("KERNEL_BF16", "1") == "1"
STAGED = 1  # how many trailing items use the staged/split tail form
XDT = BF if USE_BF16 else FPR

_build_lock = threading.Lock()
_cached_nc = None


def _build():
    nc = bacc.Bacc()

    xdram_dt = BF if USE_BF16 else FP
    x1 = nc.dram_tensor("x1", [B_LOC, C, HW], xdram_dt, kind="ExternalInput")
    x2 = nc.dram_tensor("x2", [B_LOC, C, HW], xdram_dt, kind="ExternalInput")
    # wc[c, kh, o] = conv_w[o, kh*128 + c] for kh<2 (transposed lhsT layout);
    # wc[p, 2, o] = conv_w[o, C] / C (broadcast across partitions).
    wc = nc.dram_tensor("wc", [128, 3, C], BF if USE_BF16 else FP, kind="ExternalInput")
    # rs[o_part, 0, 2*itm + m] = r = sum_hw(x*attn); rs[o_part, 1, ...] = s =
    # sum_hw(attn). Cols >= 2*ITEMS are spare partial-accumulator columns
    # (zeroed; _postprocess adds them unconditionally).
    NCOL = 2 * ITEMS + 6
    rs_out = nc.dram_tensor("rs", [128, 2, NCOL], FP, kind="ExternalOutput")

    with tile.TileContext(nc) as tc, ExitStack() as ctx:
        singles = ctx.enter_context(tc.tile_pool(name="singles", bufs=1))
        xpool = ctx.enter_context(tc.tile_pool(name="xp", bufs=10))
        wpool = ctx.enter_context(tc.tile_pool(name="wp", bufs=8))
        apool = ctx.enter_context(tc.tile_pool(name="ap", bufs=6))
        # prod tiles are write-only scratch (only accum_out matters); keep
        # them in a tiny separate pool so they don't rotate the a-tiles out.
        prodpool = ctx.enter_context(tc.tile_pool(name="pp", bufs=2))
        psum = ctx.enter_context(tc.tile_pool(name="ps", bufs=4, space="PSUM"))

        wc_sb = singles.tile([128, 3, C], BF if USE_BF16 else FP)
        nc.sync.dma_start(out=wc_sb, in_=wc[:, :, :])
        rs_sb = singles.tile([128, 2, NCOL], FP)
        r_sb = rs_sb[:, 0, :]
        s_sb = rs_sb[:, 1, :]
        nc.vector.memset(rs_sb[:, :, 2 * ITEMS:], 0.0)
        # Absorb the weight-DMA wait into the consuming engines' vector
        # clocks up front, so per-item instructions don't each carry an
        # extra sync wait (walrus rejects ops with too many waits).
        absorb = singles.tile([128, 2], FP)
        nc.vector.tensor_copy(out=absorb[:, 0:1], in_=wc_sb[:, 0, 0:1])
        nc.gpsimd.tensor_copy(out=absorb[:, 1:2], in_=wc_sb[:, 0, 0:1])


        ADT = XDT if USE_BF16 else FP

        def make_weff(cen_aps, itm=None):
            # weff[c, o] = wt[c, o] + x[c, center] * wlast[o]/C, on GpSimd
            # (Pool) — otherwise idle — keeping DVE for the r-reduce. Pool
            # rejects the fused STT op, so two steps.
            weffs = []
            for kh in range(2):
                if USE_BF16:
                    # tensor_scalar requires an fp32 scalar; upconvert first.
                    # Items 0-1 sit on the pipeline lead-in: do the copy on
                    # DVE (no Q7 launch, no extra engine hop before the DVE
                    # weff that consumes it).
                    cen = wpool.tile([128, 1], FP, tag="cen")
                    if itm is not None and itm < 2:
                        nc.vector.tensor_copy(out=cen, in_=cen_aps[kh])
                    else:
                        nc.gpsimd.tensor_copy(out=cen, in_=cen_aps[kh])
                    cen_ap = cen
                else:
                    cen_ap = cen_aps[kh].bitcast(FP)
                weff = wpool.tile([128, C], XDT, tag="weff")
                if itm is not None and itm < 2:
                    # lead-in: DVE is idle until the first r-reduces arrive;
                    # the fused form cuts the weff latency ~4x vs Pool 2-op
                    nc.vector.scalar_tensor_tensor(
                        out=weff,
                        in0=wc_sb[:, 2, :],
                        scalar=cen_ap,
                        in1=wc_sb[:, kh, :],
                        op0=mybir.AluOpType.mult,
                        op1=mybir.AluOpType.add,
                    )
                else:
                    delta = wpool.tile([128, C], FP, tag="delta")
                    nc.gpsimd.tensor_scalar_mul(delta, wc_sb[:, 2, :], cen_ap)
                    nc.gpsimd.tensor_tensor(
                        out=weff, in0=delta, in1=wc_sb[:, kh, :],
                        op=mybir.AluOpType.add,
                    )
                weffs.append(weff)
            return weffs

        def sig_and_reduce(y_ap, x_ap, a_ap, prod_ap, rcol):
            nc.scalar.activation(
                out=a_ap,
                in_=y_ap,
                func=mybir.ActivationFunctionType.Sigmoid,
                accum_out=s_sb[:, rcol:rcol + 1],
            )
            # Fused multiply + free-axis reduce: r = sum_hw(a * x).
            # (InstTensorTensorReduce miscompiles on this stack; the
            # TensorScalarPtr form with accum_out works.)
            nc.vector.scalar_tensor_tensor(
                out=prod_ap,
                in0=a_ap,
                scalar=1.0,
                in1=x_ap,
                op0=mybir.AluOpType.bypass,
                op1=mybir.AluOpType.mult,
                accum_out=r_sb[:, rcol:rcol + 1],
            )

        N_STAGED = STAGED  # last N items use the staged/split form

        def staged_load(itm, src, bi):
            # Staged item: kh0 loads in full first, kh1 in two half-tiles
            # (hi half first — it holds the center column, unblocking weff);
            # each (m, n) quadrant gets its own 1-bank PSUM tile and partial
            # r/s column so every sigmoid/reduce fires as soon as ITS two
            # matmuls finish. Shortens the post-DMA tail chain.
            x0 = xpool.tile([128, HW], XDT, tag="x", name=f"xs0_{itm}")
            d0 = src[bi, 0:128, :]
            nc.sync.dma_start(out=x0, in_=d0 if USE_BF16 else d0.bitcast(FPR))
            xh = {}
            for n in (1, 0):
                xt = xpool.tile([128, 512], XDT, tag="xl", name=f"xs1_{itm}_{n}")
                dn = src[bi, 128:256, n * 512:(n + 1) * 512]
                nc.sync.dma_start(out=xt, in_=dn if USE_BF16 else dn.bitcast(FPR))
                xh[n] = xt

            # Tail weffs on split engines: weff0 on Pool (idle by the time
            # x0 lands, ahead of DVE's r-reduce backlog), weff1 fused on DVE
            # (its data lands later; DVE frees up by then). Both beat
            # waiting behind DVE's queue, which gated all 8 tail matmuls.
            caps = [x0[:, CENTER:CENTER + 1],
                    xh[1][:, CENTER - 512:CENTER - 511]]
            if USE_BF16:
                fixed = []
                for kh, cap in enumerate(caps):
                    cen = wpool.tile([128, 1], FP, tag="cen",
                                     name=f"cens_{itm}_{kh}")
                    nc.gpsimd.tensor_copy(out=cen, in_=cap)
                    fixed.append(cen)
                caps = fixed
            else:
                caps = [c.bitcast(FP) for c in caps]
            weff0 = wpool.tile([128, C], XDT, tag="weff", name=f"weffs_{itm}_0")
            delta0 = wpool.tile([128, C], FP, tag="delta", name=f"deltas_{itm}")
            nc.gpsimd.tensor_scalar_mul(delta0, wc_sb[:, 2, :], caps[0])
            nc.gpsimd.tensor_tensor(
                out=weff0, in0=delta0, in1=wc_sb[:, 0, :],
                op=mybir.AluOpType.add,
            )
            weff1 = wpool.tile([128, C], XDT, tag="weff", name=f"weffs_{itm}_1")
            nc.vector.scalar_tensor_tensor(
                out=weff1,
                in0=wc_sb[:, 2, :],
                scalar=caps[1],
                in1=wc_sb[:, 1, :],
                op0=mybir.AluOpType.mult,
                op1=mybir.AluOpType.add,
            )
            weffs = [weff0, weff1]
            return x0, xh, weffs

        def staged_compute(itm, x0, xh, weffs, part_base):
            # Full-width sigmoids (2 ops, not 4): the tail is ACT-throughput
            # bound, so fewer/bigger ACT ops beat earlier-starting halves.
            ys = {}
            for m in range(2):
                ys[m] = psum.tile([128, HW], FP, tag="y", name=f"ys_{itm}_{m}")
                for n in range(2):
                    nc.tensor.matmul(
                        out=ys[m][:, n * 512:(n + 1) * 512],
                        lhsT=weffs[0][:, m * 128:(m + 1) * 128],
                        rhs=x0[:, n * 512:(n + 1) * 512],
                        start=True,
                        stop=False,
                    )
            for n in (1, 0):
                for m in range(2):
                    nc.tensor.matmul(
                        out=ys[m][:, n * 512:(n + 1) * 512],
                        lhsT=weffs[1][:, m * 128:(m + 1) * 128],
                        rhs=xh[n],
                        start=False,
                        stop=True,
                    )
            for m in range(2):
                xin_full = x0 if m == 0 else None
                ac = apool.tile([128, HW], ADT, tag="a", name=f"as_{itm}_{m}")
                pc = prodpool.tile([128, HW], ADT, tag="prod", name=f"ps_{itm}_{m}")
                if m == 0:
                    sig_and_reduce(ys[m], x0, ac, pc, 2 * itm + m)
                else:
                    # m=1's multiplier lives in two half-tiles: sigmoid once,
                    # then two half-width reduces into base + partial cols
                    nc.scalar.activation(
                        out=ac, in_=ys[m],
                        func=mybir.ActivationFunctionType.Sigmoid,
                        accum_out=s_sb[:, 2 * itm + m:2 * itm + m + 1],
                    )
                    for n, rcol in ((0, 2 * itm + m), (1, part_base + m)):
                        nc.vector.scalar_tensor_tensor(
                            out=pc[:, n * 512:(n + 1) * 512],
                            in0=ac[:, n * 512:(n + 1) * 512],
                            scalar=1.0,
                            in1=xh[n],
                            op0=mybir.AluOpType.bypass,
                            op1=mybir.AluOpType.mult,
                            accum_out=r_sb[:, rcol:rcol + 1],
                        )

        assert N_STAGED == 1

        def emit_item(itm):
            src = x1 if itm < B_LOC else x2
            bi = itm % B_LOC

            xs = []
            for kh in range(2):
                xt = xpool.tile([128, HW], XDT, tag="x", name=f"x_{itm}_{kh}")
                din = src[bi, kh * 128:(kh + 1) * 128, :]
                nc.sync.dma_start(out=xt, in_=din if USE_BF16 else din.bitcast(FPR))
                xs.append(xt)

            weffs = make_weff([x[:, CENTER:CENTER + 1] for x in xs], itm=itm)

            for m in range(2):  # output-channel halves
                y = psum.tile([128, HW], FP, tag="y", name=f"y_{itm}_{m}")
                for n, kh in [(0, 0), (1, 0), (0, 1), (1, 1)]:  # kh-outer: LDW reuse
                    nc.tensor.matmul(
                        out=y[:, n * 512:(n + 1) * 512],
                        lhsT=weffs[kh][:, m * 128:(m + 1) * 128],
                        rhs=xs[kh][:, n * 512:(n + 1) * 512],
                        start=(kh == 0),
                        stop=(kh == 1),
                    )
                a = apool.tile([128, HW], ADT, tag="a", name=f"a_{itm}_{m}")
                prod = prodpool.tile([128, HW], ADT, tag="prod", name=f"p_{itm}_{m}")
                sig_and_reduce(y, xs[m], a, prod, 2 * itm + m)

        # Emit item 0 first (lead-in latency), then the staged tail item's
        # LOADS (so its x tiles are on-chip long before the tail and its
        # DVE-weff never stalls the queued r-reduces), then the rest.
        emit_item(0)
        staged_ctx = staged_load(ITEMS - 1, x2, B_LOC - 1)
        for itm in range(1, ITEMS - 1):
            emit_item(itm)
        staged_compute(ITEMS - 1, *staged_ctx, 2 * ITEMS)

        # Split the result writeback: items 0-13 go out as soon as their
        # reduces land (hidden under the tail compute); only the last two
        # items + staged partials ride the final, latency-dominated DMA.
        SPLIT = 2 * (ITEMS - 2)
        nc.sync.dma_start(out=rs_out[:, :, 0:SPLIT], in_=rs_sb[:, :, 0:SPLIT])
        nc.sync.dma_start(out=rs_out[:, :, SPLIT:], in_=rs_sb[:, :, SPLIT:])

    nc.finalize()
    return nc


def _get_nc():
    global _cached_nc
    with _build_lock:
        if _cached_nc is None:
            _cached_nc = _build()
    return _cached_nc


def _make_in_maps(x1, x2, conv_w):
    conv_w = np.asarray(conv_w, dtype=np.float32)
    if USE_BF16:
        import ml_dtypes
        x1r = np.asarray(x1, dtype=np.float32).reshape(B, C, HW).astype(ml_dtypes.bfloat16)
        x2r = np.asarray(x2, dtype=np.float32).reshape(B, C, HW).astype(ml_dtypes.bfloat16)
    else:
        x1r = np.ascontiguousarray(x1, dtype=np.float32).reshape(B, C, HW)
        x2r = np.ascontiguousarray(x2, dtype=np.float32).reshape(B, C, HW)
    wcomb = np.empty((128, 3, C), np.float32)
    wcomb[:, 0:2, :] = conv_w[:, :C].T.reshape(2, 128, C).transpose(1, 0, 2)
    wcomb[:, 2, :] = conv_w[:, C] / C
    if USE_BF16:
        import ml_dtypes
        wcomb = wcomb.astype(ml_dtypes.bfloat16)
    return [
        {
            "x1": x1r[c * B_LOC:(c + 1) * B_LOC],
            "x2": x2r[c * B_LOC:(c + 1) * B_LOC],
            "wc": wcomb,
        }
        for c in range(N_CORES)
    ]


def _postprocess(results):
    out = np.empty((B, C), np.float32)
    for c in range(N_CORES):
        rs = results[c]["rs"]
        r = rs[:, 0, :2 * ITEMS].copy()
        s = rs[:, 1, :2 * ITEMS].copy()
        # fold staged items' partial-accumulator columns back in
        for k in range(STAGED):
            itm = ITEMS - STAGED + k
            for m in range(2):
                r[:, 2 * itm + m] += rs[:, 0, 2 * ITEMS + 2 * k + m]
                s[:, 2 * itm + m] += rs[:, 1, 2 * ITEMS + 2 * k + m]
        # r[o, 2*itm + m] -> pool[itm, m*128 + o]
        pool_rs = (r / (s + 1e-8)).reshape(128, ITEMS, 2)
        pool = np.transpose(pool_rs, (1, 2, 0)).reshape(ITEMS, C)
        out[c * B_LOC:(c + 1) * B_LOC] = pool[:B_LOC] + pool[B_LOC:]
    return out


def _run(x1, x2, conv_w, **bass_kwargs):
    nc = _get_nc()
    in_maps = _make_in_maps(x1, x2, conv_w)
    res = run_bass_kernel_spmd(nc, in_maps, list(range(N_CORES)), **bass_kwargs)
    return _postprocess(res.results), res


def kernel(x1, x2, conv_w, ca_w1=None, ca_b1=None, ca_w2=None, ca_b2=None):
    out, _ = _run(x1, x2, conv_w)
    return out

